# revision 5
# baseline (speedup 1.0000x reference)
"""Multi-head attention (N=2, K=2048, DIN=1024, H=16, DOUT=64) on 8 TRN2 NeuronCores.

Sharding: data-parallel over batch N (cores 0-3 -> n=0, cores 4-7 -> n=1),
tensor-parallel over heads (4 heads per core). Each core computes its 4 heads'
attention plus the partial output projection over its head-feature slice of Wp;
the host sums the 4 bf16 partials per batch element and adds the output bias.

v2 schedule (from baseline trace analysis: PE busy 85.6% but ~50us of it was
filler, lead-in 48.6us, exp-stream gaps 48us, tail 31.6us):
  - xq/xk are DMA'd in COLUMN slices (512 cols x all 8 DIN-subtiles) so the
    q/k projection chunks trail the DMA chunk-by-chunk; only the minimal
    attention-start set (xq cols 0-1023, xk cols 0-511) is issued upfront so
    the DMA rings don't spread bandwidth across the whole 8MB load.  The
    remaining column slices are issued from inside the early kt iterations.
  - attention starts after qp0(c0,c1)+kp0(c0) (~17us instead of 48us); the
    remaining projection chunks (kp0 c1-3, qp0 c2-3, qp1, kp1) are injected
    into the kt stream ahead of their consuming block.
  - S matmuls for the two heads of a pair are issued interleaved so their
    disjoint row-groups (K=64 each) run concurrently on the PE.
  - epilogue flushes live on kts 2,3,4,6 of the next block; output-projection
    groups on kts 5,7-15 of block 4; DVE per-kt load stays under the 2us exp
    pace so the exp stream doesn't stall on the pt-slot WAR (pt bufs=4).
  - tail: flushes interleaved with per-st merged output groups ([128,1024]
    ost + a single DMA per st) with psum->SBUF drains split ACT/DVE.
"""

import numpy as np
import ml_dtypes

import concourse.bass as bass
import concourse.mybir as mybir
from concourse import bacc
from concourse.tile import TileContext

P = 128
SEQ = 2048
DIN = 1024
DOUT = 64
H = 16
N = 2
HPC = 4  # heads per core
NPAIR = 2  # head pairs per core
KSUB = DIN // P  # 8 contraction subtiles for projections
NKT = SEQ // P  # 16 seq_k tiles of 128
NC = 4  # x column chunks of 512
BF = mybir.dt.bfloat16
F32 = mybir.dt.float32
BF_NP = ml_dtypes.bfloat16

_NC_CACHE = None


def build_bass():
    nc = bacc.Bacc()

    xq_d = nc.declare_dram_parameter("xqT", [DIN, SEQ], BF, isOutput=False)
    xk_d = nc.declare_dram_parameter("xkT", [DIN, SEQ], BF, isOutput=False)
    xv_d = nc.declare_dram_parameter("xvb", [P, NKT, KSUB * P], BF, isOutput=False)
    mk_d = nc.declare_dram_parameter("maskT", [SEQ, SEQ], BF, isOutput=False)
    wq_d = nc.declare_dram_parameter("wq", [DIN, HPC * DOUT], BF, isOutput=False)
    wk_d = nc.declare_dram_parameter("wk", [DIN, HPC * DOUT], BF, isOutput=False)
    wv_d = nc.declare_dram_parameter("wv", [DIN, HPC * DOUT], BF, isOutput=False)
    wp_d = nc.declare_dram_parameter("wp", [HPC * DOUT, DIN], BF, isOutput=False)
    bqp_d = nc.declare_dram_parameter("bqp", [P, NPAIR], F32, isOutput=False)
    bkp_d = nc.declare_dram_parameter("bkp", [P, NPAIR], F32, isOutput=False)
    bvr_d = nc.declare_dram_parameter("bvr", [P, HPC * DOUT], F32, isOutput=False)
    out_d = nc.declare_dram_parameter("out", [SEQ, DIN], BF, isOutput=True)

    ADD = mybir.AluOpType.add
    EXP = mybir.ActivationFunctionType.Exp

    xq_v = xq_d.rearrange("(o p) s -> p o s", p=P)
    xk_v = xk_d.rearrange("(o p) s -> p o s", p=P)

    with TileContext(nc) as tc:
        with (
            tc.tile_pool(name="const", bufs=1) as const,
            tc.tile_pool(name="xin", bufs=1) as xin,
            tc.tile_pool(name="proj", bufs=1) as proj,
            tc.tile_pool(name="maskp", bufs=3) as maskp,
            tc.tile_pool(name="ptp", bufs=4) as ptp,
            tc.tile_pool(name="epi", bufs=2) as epi,
            tc.tile_pool(name="ps_s", bufs=2, space="PSUM") as ps_s,
            tc.tile_pool(name="ps_pv", bufs=2, space="PSUM") as ps_pv,
            tc.tile_pool(name="ps_dn", bufs=1, space="PSUM") as ps_dn,
            tc.tile_pool(name="ps_aux", bufs=1, space="PSUM") as ps_aux,
        ):
            # ---- constants --------------------------------------------------
            wq_sb = const.tile([P, KSUB, HPC * DOUT], BF)
            nc.sync.dma_start(wq_sb[:], wq_d.rearrange("(o p) m -> p o m", p=P))
            wk_sb = const.tile([P, KSUB, HPC * DOUT], BF)
            nc.sync.dma_start(wk_sb[:], wk_d.rearrange("(o p) m -> p o m", p=P))
            wv_sb = const.tile([P, KSUB, HPC * DOUT], BF)
            nc.sync.dma_start(wv_sb[:], wv_d.rearrange("(o p) m -> p o m", p=P))
            bqp_sb = const.tile([P, NPAIR], F32)
            nc.sync.dma_start(bqp_sb[:], bqp_d[:])
            bkp_sb = const.tile([P, NPAIR], F32)
            nc.sync.dma_start(bkp_sb[:], bkp_d[:])
            bvr_sb = const.tile([P, HPC * DOUT], F32)
            nc.sync.dma_start(bvr_sb[:], bvr_d[:])

            ones_sb = const.tile([P, 1], BF)
            nc.vector.memset(ones_sb[:], 1.0)
            ones64_sb = const.tile([P, DOUT], BF)
            nc.vector.memset(ones64_sb[:], 1.0)
            warm_rhs = const.tile([P, 512], BF)
            nc.vector.memset(warm_rhs[:], 1.0)
            exp_warm = const.tile([1, 4], F32)
            nc.scalar.activation(exp_warm[:], warm_rhs[0:1, 0:4], EXP)

            # ---- activations: minimal attention-start set first -------------
            xq_sb = xin.tile([P, KSUB, SEQ], BF)
            xk_sb = xin.tile([P, KSUB, SEQ], BF)

            def dma_x(x_sb, x_v, c):
                nc.sync.dma_start(
                    x_sb[:, :, c * 512 : (c + 1) * 512],
                    x_v[:, :, c * 512 : (c + 1) * 512],
                )

            dma_x(xq_sb, xq_v, 0)
            dma_x(xq_sb, xq_v, 1)
            dma_x(xk_sb, xk_v, 0)

            xv_sb = xin.tile([P, NKT, KSUB, P], BF)

            def dma_xv_st(st):
                nc.sync.dma_start(xv_sb[:, st, :, :], xv_d[:, st, :])

            for st in range(3):
                dma_xv_st(st)

            pre_masks = []
            for kt in range(2):
                mt = maskp.tile([P, 1024], BF, tag="mt", name="mt")
                nc.sync.dma_start(mt[:], mk_d[kt * P : (kt + 1) * P, 0:1024])
                pre_masks.append(mt)

            wp_sb = const.tile([P, NPAIR, DIN], BF)

            def filler(n, cols=256):
                # dependency-free PE work bridging stalls so the HAM clock
                # gate never re-throttles across a DMA-wait window.
                fps = ps_aux.tile([P, 512], F32, tag="aux", name="fps")
                for _ in range(n):
                    nc.tensor.matmul(
                        fps[64:P, 256 : 256 + cols],
                        warm_rhs[64:P, 0:64],
                        warm_rhs[64:P, 0:cols],
                        start=True,
                        stop=True,
                        skip_group_check=True,
                    )

            def warm_fill(n):
                warm_ps = ps_s.tile([P, 1024], F32, tag="s", name="warm_ps")
                for _ in range(n):
                    nc.tensor.matmul(
                        warm_ps[:, 0:256],
                        warm_rhs[:, 0:P],
                        warm_rhs[:, 0:256],
                        start=True,
                        stop=True,
                        skip_group_check=True,
                    )

            # ---- persistent intermediates -----------------------------------
            qhT = proj.tile([P, NPAIR, SEQ], BF)
            khT = proj.tile([P, NPAIR, SEQ], BF)
            vsb = proj.tile([P, NKT, HPC * DOUT], BF)
            ynT = proj.tile([P, NPAIR, SEQ], BF)
            vsb4 = vsb.rearrange("p k (h c) -> p k h c", c=DOUT)

            # ---- projections ------------------------------------------------
            def proj_chunk(w_sb, x_sb, b_sb, o_sb, pair, c0, cols, pool=None):
                if pool is None:
                    pps = ps_pv.tile([P, 512], F32, tag="pv", name="pps")
                else:
                    pps = pool.tile([P, 512], F32, tag="aux", name="ppsa")
                for o in range(KSUB):
                    nc.tensor.matmul(
                        pps[:, :cols],
                        w_sb[:, o, pair * P : (pair + 1) * P],
                        x_sb[:, o, c0 : c0 + cols],
                        start=(o == 0),
                        stop=(o == KSUB - 1),
                    )
                nc.vector.tensor_tensor(
                    o_sb[:, pair, c0 : c0 + cols],
                    pps[:, :cols],
                    b_sb[:, pair : pair + 1].to_broadcast((P, cols)),
                    ADD,
                )

            def proj_v_st(st):
                vps = ps_aux.tile([P, 512], F32, tag="aux", name="vps")
                for o in range(KSUB):
                    nc.tensor.matmul(
                        vps[:, : HPC * DOUT],
                        xv_sb[:, st, o, :],
                        wv_sb[:, o, :],
                        start=(o == 0),
                        stop=(o == KSUB - 1),
                    )
                nc.vector.tensor_tensor(
                    vsb[:, st, :], vps[:, : HPC * DOUT], bvr_sb[:], ADD
                )

            # ---- epilogue machinery -----------------------------------------
            pending = []

            def flush_epilogue(items):
                for rcp_sb, rcp96, pv_sbs, pair_, qh_, qi, h2 in items:
                    q0 = (qh_ * 2 + qi) * 512
                    row = 32 * qi + 64 * h2
                    rdb = ps_aux.tile([P, 512], F32, tag="aux", name="rdb")
                    src, srow = (rcp96, 0) if row == 96 else (rcp_sb, row)
                    nc.tensor.matmul(
                        rdb[0:DOUT, :],
                        ones64_sb[srow : srow + 1, :],
                        src[srow : srow + 1, :],
                        start=True,
                        stop=True,
                        skip_group_check=True,
                    )
                    nc.vector.tensor_mul(
                        ynT[h2 * DOUT : (h2 + 1) * DOUT, pair_, q0 : q0 + 512],
                        pv_sbs[qi][h2 * DOUT : (h2 + 1) * DOUT, :],
                        rdb[0:DOUT, :],
                    )

            class BlockState:
                def __init__(self, pair, qh):
                    self.pair, self.qh = pair, qh
                    self.pvs = []
                    for i in range(2):
                        pv = ps_pv.tile([P, 512], F32, tag="pv", name=f"pv{i}")
                        self.pvs.append(pv)
                    self.dn = ps_dn.tile([P, 512], F32, tag="dn", name="dn")
                    nc.vector.memset(self.dn[:], 0.0)

            def pv_dn_mms(bs, kt, ptms):
                for qi in range(2):
                    for h2 in range(2):
                        nc.tensor.matmul(
                            bs.pvs[qi][h2 * DOUT : (h2 + 1) * DOUT, :],
                            vsb4[:, kt, bs.pair * 2 + h2, :],
                            ptms[h2][:, qi * 512 : (qi + 1) * 512],
                            start=(kt == 0),
                            stop=(kt == NKT - 1),
                            tile_position=(0, h2 * DOUT),
                            skip_group_check=True,
                        )
                for qi in range(2):
                    for h2 in range(2):
                        row = 32 * qi + 64 * h2
                        nc.tensor.matmul(
                            bs.dn[row : row + 1, :],
                            ones_sb[:],
                            ptms[h2][:, qi * 512 : (qi + 1) * 512],
                            start=False,
                            stop=(kt == NKT - 1),
                            tile_position=(0, row),
                            skip_group_check=True,
                        )

            def block_epilogue(bs):
                rcp_f = epi.tile([P, 512], F32, tag="rcpf", name="rcp_f", bufs=2)
                nc.vector.reciprocal_approx_fast(rcp_f[:], bs.dn[:])
                rcp_sb = epi.tile([P, 512], BF, tag="rcpb", name="rcp_sb", bufs=2)
                nc.vector.tensor_copy(rcp_sb[:], rcp_f[:])
                rcp96 = epi.tile([1, 512], BF, tag="rcp96", name="rcp96", bufs=2)
                nc.vector.tensor_copy(rcp96[:], rcp_f[96:97, :])
                pv_sbs = []
                for qi in range(2):
                    pv_sb = epi.tile([P, 512], F32, tag="pvsb", name="pv_sb", bufs=4)
                    nc.vector.tensor_copy(pv_sb[:], bs.pvs[qi][:])
                    pv_sbs.append(pv_sb)
                for qi in range(2):
                    for h2 in range(2):
                        pending.append(
                            (rcp_sb, rcp96, pv_sbs, bs.pair, bs.qh, qi, h2)
                        )

            # ---- output projection ------------------------------------------
            def phase_c_group(st, ntile):
                # injected (in-block) C group: aux psum, pure-DVE drain
                cps = ps_aux.tile([P, 512], F32, tag="aux", name="cpsa")
                for pair in range(NPAIR):
                    nc.tensor.matmul(
                        cps[:],
                        ynT[:, pair, st * P : (st + 1) * P],
                        wp_sb[:, pair, ntile * 512 : (ntile + 1) * 512],
                        start=(pair == 0),
                        stop=(pair == NPAIR - 1),
                    )
                ost = epi.tile([P, 512], BF, tag="ost", name="ost", bufs=3)
                nc.vector.tensor_copy(ost[:], cps[:])
                nc.sync.dma_start(
                    out_d[st * P : (st + 1) * P, ntile * 512 : (ntile + 1) * 512],
                    ost[:],
                )

            def phase_c_st(st):
                # tail C group: both ntiles, merged ost + single DMA,
                # drains split across ACT and DVE (both idle in the tail)
                ost = epi.tile([P, 1024], BF, tag="ost2", name="ost2", bufs=2)
                for ntile in range(2):
                    if ntile == 0:
                        cps = ps_pv.tile([P, 512], F32, tag="pv", name="cps")
                    else:
                        cps = ps_s.tile([P, 1024], F32, tag="s", name="cps2")[:, :512]
                    for pair in range(NPAIR):
                        nc.tensor.matmul(
                            cps[:],
                            ynT[:, pair, st * P : (st + 1) * P],
                            wp_sb[:, pair, ntile * 512 : (ntile + 1) * 512],
                            start=(pair == 0),
                            stop=(pair == NPAIR - 1),
                        )
                    o = ntile * 512
                    nc.scalar.copy(ost[:, o : o + 256], cps[:, :256])
                    nc.vector.tensor_copy(ost[:, o + 256 : o + 512], cps[:, 256:])
                nc.sync.dma_start(out_d[st * P : (st + 1) * P, :], ost[:])

            # ---- per-kt injection schedule (linear it over 4 blocks) --------
            schedule = [(0, 0), (0, 1), (1, 0), (1, 1)]

            inject_pe = {}  # it -> list of thunks (PE work, emitted pre-S)
            inject_dma = {}  # it -> list of thunks (DMA issue, emitted pre-S)

            def add(d, it, fn):
                d.setdefault(it, []).append(fn)

            # block 1: v projection (st kt+1), trailing x-chunk DMAs and the
            # remaining kp0 / qp0 chunks as their slices land
            add(inject_pe, 0, lambda: proj_v_st(0))
            add(inject_pe, 0, lambda: proj_v_st(1))
            for j in range(1, NKT - 1):
                add(inject_pe, j, lambda st=j + 1: proj_v_st(st))
            for it in range(NKT - 3):
                add(inject_dma, it, lambda st=it + 3: dma_xv_st(st))
            add(inject_dma, 0, lambda: dma_x(xk_sb, xk_v, 1))
            add(inject_dma, 1, lambda: dma_x(xk_sb, xk_v, 2))
            add(inject_dma, 2, lambda: dma_x(xk_sb, xk_v, 3))
            add(inject_dma, 2, lambda: dma_x(xq_sb, xq_v, 2))
            add(inject_dma, 3, lambda: dma_x(xq_sb, xq_v, 3))
            add(inject_dma, 4, lambda: nc.sync.dma_start(
                wp_sb[:], wp_d.rearrange("(o p) n -> p o n", p=P)))
            for it, c in ((3, 1), (7, 2), (11, 3)):
                add(inject_pe, it,
                    lambda c=c: proj_chunk(wk_sb, xk_sb, bkp_sb, khT, 0,
                                           c * 512, 512, pool=ps_aux))
            add(inject_pe, 13,
                lambda: proj_chunk(wq_sb, xq_sb, bqp_sb, qhT, 0, 2 * 512, 512,
                                   pool=ps_aux))
            add(inject_pe, 15,
                lambda: proj_chunk(wq_sb, xq_sb, bqp_sb, qhT, 0, 3 * 512, 512,
                                   pool=ps_aux))
            # block 2: qp1 + kp1 c0,c1
            for it, c in ((17, 0), (21, 1), (25, 2), (29, 3)):
                add(inject_pe, it,
                    lambda c=c: proj_chunk(wq_sb, xq_sb, bqp_sb, qhT, 1,
                                           c * 512, 512, pool=ps_aux))
            for it, c in ((23, 0), (27, 1)):
                add(inject_pe, it,
                    lambda c=c: proj_chunk(wk_sb, xk_sb, bkp_sb, khT, 1,
                                           c * 512, 512, pool=ps_aux))
            # block 3: kp1 c2,c3
            for it, c in ((33, 2), (37, 3)):
                add(inject_pe, it,
                    lambda c=c: proj_chunk(wk_sb, xk_sb, bkp_sb, khT, 1,
                                           c * 512, 512, pool=ps_aux))
            # block 4: output projection groups (st0-6, both ntiles)
            c_sched = {
                5: [(0, 0)], 7: [(0, 1)], 8: [(1, 0), (1, 1)],
                9: [(2, 0)], 10: [(2, 1), (3, 0)], 11: [(3, 1)],
                12: [(4, 0), (4, 1)], 13: [(5, 0)],
                14: [(5, 1), (6, 0)], 15: [(6, 1)],
            }
            for kt, groups in c_sched.items():
                for st, ntile in groups:
                    add(inject_pe, 48 + kt,
                        lambda st=st, ntile=ntile: phase_c_group(st, ntile))

            FLUSH_KTS = (2, 3, 4, 6)

            # ---- attention ---------------------------------------------------
            def attn_blocks(premask):
                prev = None
                bs = None
                for it in range(len(schedule) * NKT):
                    b, kt = divmod(it, NKT)
                    pair, qh = schedule[b]
                    if it == 0:
                        bs = BlockState(pair, qh)
                    injected = False
                    for fn in inject_dma.get(it, ()):
                        fn()
                    if pending and kt in FLUSH_KTS:
                        flush_epilogue([pending.pop(0)])
                        injected = True
                    for fn in inject_pe.get(it, ()):
                        fn()
                        injected = True
                    if not injected:
                        filler(2)
                    if b == 0 and kt < len(premask):
                        mt = premask[kt]
                    else:
                        mt = maskp.tile([P, 1024], BF, tag="mt", name="mt")
                        nc.sync.dma_start(
                            mt[:],
                            mk_d[kt * P : (kt + 1) * P, qh * 1024 : (qh + 1) * 1024],
                        )
                    # S matmuls: h2 pairs interleaved -> disjoint row groups
                    # (K=64 at partitions 0-63 / 64-127) run concurrently
                    sps = [
                        ps_s.tile([P, 1024], F32, tag="s", name="sps")
                        for _ in range(2)
                    ]
                    for qi in range(2):
                        for h2 in range(2):
                            hs = slice(h2 * DOUT, (h2 + 1) * DOUT)
                            q0 = (qh * 2 + qi) * 512
                            nc.tensor.matmul(
                                sps[h2][:, qi * 512 : (qi + 1) * 512],
                                khT[hs, pair, kt * P : (kt + 1) * P],
                                qhT[hs, pair, q0 : q0 + 512],
                                start=True,
                                stop=True,
                                skip_group_check=True,
                            )
                    ptms = []
                    for h2 in range(2):
                        pt = ptp.tile([P, 1024], BF, tag=f"pt{h2}", name="pt")
                        nc.scalar.activation(pt[:], sps[h2][:], EXP, scale=0.125)
                        ptm = ptp.tile(
                            [P, 1024], BF, tag=f"ptm{h2}", name="ptm", bufs=3
                        )
                        nc.vector.tensor_mul(ptm[:], pt[:], mt[:])
                        ptms.append(ptm)
                    if prev is not None:
                        pbs, pkt, pptms = prev
                        pv_dn_mms(pbs, pkt, pptms)
                        if pkt == NKT - 1:
                            block_epilogue(pbs)
                            bs = BlockState(pair, qh)
                    prev = (bs, kt, ptms)
                pbs, pkt, pptms = prev
                pv_dn_mms(pbs, pkt, pptms)
                block_epilogue(pbs)

            # ---- phase order -------------------------------------------------
            warm_fill(75)
            proj_chunk(wq_sb, xq_sb, bqp_sb, qhT, 0, 0, 512)
            filler(6)
            proj_chunk(wq_sb, xq_sb, bqp_sb, qhT, 0, 512, 512)
            filler(6)
            proj_chunk(wk_sb, xk_sb, bkp_sb, khT, 0, 0, 512)

            attn_blocks(pre_masks)

            # tail: flushes interleaved with the remaining output groups
            flush_epilogue(pending[:2])
            phase_c_group(7, 0)
            phase_c_group(7, 1)
            for st in range(8, 12):
                phase_c_st(st)
            flush_epilogue(pending[2:])
            pending.clear()
            for st in range(12, NKT):
                phase_c_st(st)

    nc.finalize()
    return nc


def make_in_maps(query, key, value, mask, Wq, bq, Wk, bk, Wv, bv, Wp, bp):
    """Shard + pre-layout the full inputs into 8 per-core input dicts."""
    in_maps = []
    for c in range(8):
        n = c // 4
        h0 = HPC * (c % 4)
        hs = slice(h0, h0 + HPC)

        def t_bf(x):  # [SEQ, DIN] -> contiguous [DIN, SEQ] bf16
            return np.ascontiguousarray(x.T).astype(BF_NP)

        def w_bf(W):
            return np.ascontiguousarray(
                W[hs].transpose(1, 0, 2).reshape(DIN, HPC * DOUT)
            ).astype(BF_NP)

        def b_pair(b):
            return np.ascontiguousarray(b[hs].reshape(NPAIR, P).T).astype(np.float32)

        xvT = value[n].T.astype(BF_NP)  # [DIN, SEQ]
        xvb = np.ascontiguousarray(
            xvT.reshape(KSUB, P, NKT, P).transpose(1, 2, 0, 3).reshape(P, NKT, KSUB * P)
        )

        in_maps.append(
            {
                "xqT": t_bf(query[n]),
                "xkT": t_bf(key[n]),
                "xvb": xvb,
                "maskT": np.ascontiguousarray((~mask[n]).T).astype(BF_NP),
                "wq": w_bf(Wq),
                "wk": w_bf(Wk),
                "wv": w_bf(Wv),
                "wp": np.ascontiguousarray(
                    Wp[h0 * DOUT : (h0 + HPC) * DOUT, :]
                ).astype(BF_NP),
                "bqp": b_pair(bq),
                "bkp": b_pair(bk),
                "bvr": np.ascontiguousarray(
                    np.tile(bv[hs].reshape(1, HPC * DOUT), (P, 1))
                ).astype(np.float32),
            }
        )
    return in_maps


def kernel(**inputs):
    global _NC_CACHE
    from concourse.bass_utils import run_bass_kernel_spmd

    if _NC_CACHE is None:
        _NC_CACHE = build_bass()
    nc = _NC_CACHE

    in_maps = make_in_maps(**inputs)
    res = run_bass_kernel_spmd(nc, in_maps, core_ids=list(range(8))).results
    parts = [res[c]["out"].astype(np.float32) for c in range(8)]
    bp = inputs["bp"]
    out = np.stack(
        [
            parts[0] + parts[1] + parts[2] + parts[3] + bp[None, :],
            parts[4] + parts[5] + parts[6] + parts[7] + bp[None, :],
        ]
    )
    return out.astype(np.float32)
